# revision 9
# baseline (speedup 1.0000x reference)
"""Trainium2 Bass kernel for nn_GroupGenerator (n=1024, 8 NeuronCores).

Math restructure (exact):
  The reference's 1x1-conv stack over diff[c,i,j] = v_abs[c,i] - v_abs[c,j]
  collapses: pre-ReLU h[o,i,j] = a[o,i] - a[o,j] + b1[o] with a = W1 @ v_abs
  (a is [32, 1024]).  BatchNorm folds into per-channel scale/bias, so

    d_pre[i,j] = sum_o wt[o] * relu(a[o,i] - a[o,j] + b1[o]) + C
    dist       = 0.5 * (exp(d_pre) + exp(d_pre)^T)

  with wt[o] = W2[o]*gamma[o]*rsqrt(var[o]+eps),
  C = sum_o W2[o]*beta[o] + b2 - sum_o wt[o]*mu[o]  (ln(0.5) folded in so the
  0.5 symmetrization becomes a plain add of the two exp() terms).

Sharding: rows of the n x n map, 128 per core.  Each core receives v_abs with
columns rolled by -128*core so its row block is always columns 0:128 (static
addressing); the host rolls each core's 128 output rows back.  BN statistics
are global: cores compute partial (sum h, sum h^2) via accum_out, reduced
across cores with a tiny [32,2] AllReduce.

Device layout: tiles are [128, 1024] with partition p = 32*r + o (r = row
within a 4-row group, o = channel).  One tensor_scalar per 4-row group
produces all 32 channels' relu values; a [128,4] block lhsT matmul contracts
channels into PSUM rows 4t+r, accumulating the full [128, 1024] d_pre per
direction across the 32 groups.  The transposed direction reuses the same
tiles' algebra with swapped operands (relu(a_j - a_i + b1)).

The sequential label-merging scan (O(n^2) int ops) runs on host, replicated
from the reference semantics exactly.

v_out: with hard=True the forward value is stop_gradient(v - v_soft) + v_soft
== v up to one fp32 rounding (~1e-7), so v is returned directly.
"""

import sys

import numpy as np

try:
    import concourse.bass as bass  # noqa: F401
except ImportError:  # pragma: no cover
    sys.path.insert(0, "/opt/trn_rl_repo")

import concourse.bacc as bacc
import concourse.bass as bass
import concourse.tile as tile
from concourse import mybir
from concourse.bass_utils import run_bass_kernel_spmd

N = 1024
NCORES = 8
RPC = N // NCORES  # rows per core = 128
G = RPC // 4  # 4-row groups per core = 32
EPS = 1e-5
TH = 1.0
FP = mybir.dt.float32
AX = mybir.AxisListType
ALU = mybir.AluOpType
AF = mybir.ActivationFunctionType

_CACHE = {}


def _build(nreps=1, debug=False, cc_cores=NCORES):
    nc = bacc.Bacc(
        "TRN2",
        target_bir_lowering=False,
        debug=False,
        enable_asserts=True,
        num_devices=cc_cores,
    )
    va_d = nc.dram_tensor("va", [16, N], FP, kind="ExternalInput").ap()
    w1t_d = nc.dram_tensor("w1t", [16, 32], FP, kind="ExternalInput").ap()
    b1r_d = nc.dram_tensor("b1rep", [128, 1], FP, kind="ExternalInput").ap()
    w2g_d = nc.dram_tensor("w2g", [32, 1], FP, kind="ExternalInput").ap()
    osel_d = nc.dram_tensor("osel", [128, 32], FP, kind="ExternalInput").ap()
    ones32_d = nc.dram_tensor("ones32", [32, 1], FP, kind="ExternalInput").ap()
    ones1x128_d = nc.dram_tensor("ones1x128", [1, 128], FP, kind="ExternalInput").ap()
    cb_d = nc.dram_tensor("cb", [1, 1], FP, kind="ExternalInput").ap()
    dist_d = nc.dram_tensor("dist", [RPC, N], FP, kind="ExternalOutput").ap()
    dbg = None if not debug else {
        "dbg_a": nc.dram_tensor("dbg_a", [32, N], FP, kind="ExternalOutput").ap(),
        "dbg_S": nc.dram_tensor("dbg_S", [128, G], FP, kind="ExternalOutput").ap(),
        "dbg_S2": nc.dram_tensor("dbg_S2", [128, G], FP, kind="ExternalOutput").ap(),
        "dbg_red2": nc.dram_tensor("dbg_red2", [128, 1], FP, kind="ExternalOutput").ap(),
        "dbg_st": nc.dram_tensor("dbg_st", [32, 2], FP, kind="ExternalOutput").ap(),
        "dbg_wt": nc.dram_tensor("dbg_wt", [32, 1], FP, kind="ExternalOutput").ap(),
        "dbg_C": nc.dram_tensor("dbg_C", [128, 1], FP, kind="ExternalOutput").ap(),
        "dbg_at0": nc.dram_tensor("dbg_at0", [128, N], FP, kind="ExternalOutput").ap(),
        "dbg_d2t0": nc.dram_tensor("dbg_d2t0", [128, N], FP, kind="ExternalOutput").ap(),
        "dbg_d1": nc.dram_tensor("dbg_d1", [128, N], FP, kind="ExternalOutput").ap(),
        "dbg_d2": nc.dram_tensor("dbg_d2", [128, N], FP, kind="ExternalOutput").ap(),
        "dbg_L": nc.dram_tensor("dbg_L", [128, 252], FP, kind="ExternalOutput").ap(),
    }

    with tile.TileContext(nc) as tc:
        for _ in range(nreps):
            _body(tc, va_d, w1t_d, b1r_d, w2g_d, osel_d, ones32_d, ones1x128_d,
                  cb_d, dist_d, dbg, cc_cores=cc_cores)
    nc.compile()
    return nc


def _body(tc, va_d, w1t_d, b1r_d, w2g_d, osel_d, ones32_d, ones1x128_d, cb_d, dist_d, dbg=None, cc_cores=NCORES):
    nc = tc.nc
    with (
        tc.tile_pool(name="const", bufs=1) as cpool,
        tc.tile_pool(name="atiles", bufs=G) as apool,
        tc.tile_pool(name="sq", bufs=2) as sqpool,
        tc.tile_pool(name="d2", bufs=6) as d2pool,
        tc.tile_pool(name="outs", bufs=1) as opool,
        tc.tile_pool(name="ps_small", bufs=2, space="PSUM") as ps_small,
        tc.tile_pool(name="ps_stat", bufs=1, space="PSUM") as ps_stat,
        tc.tile_pool(name="ps_main", bufs=1, space="PSUM") as ps_main,
        tc.tile_pool(name="dram", bufs=1, space="DRAM") as dpool,
    ):
        # ---- load inputs ----
        va = cpool.tile([16, N], FP, tag="va")
        w1t = cpool.tile([16, 32], FP, tag="w1t")
        b1r = cpool.tile([128, 1], FP, tag="b1r")
        w2g = cpool.tile([32, 1], FP, tag="w2g")
        osel = cpool.tile([128, 32], FP, tag="osel")
        ones32 = cpool.tile([32, 1], FP, tag="ones32")
        ones1x128 = cpool.tile([1, 128], FP, tag="ones1x128")
        cb = cpool.tile([1, 1], FP, tag="cb")
        nc.sync.dma_start(va[:], va_d)
        nc.sync.dma_start(w1t[:], w1t_d)
        nc.sync.dma_start(b1r[:], b1r_d)
        nc.sync.dma_start(w2g[:], w2g_d)
        nc.sync.dma_start(osel[:], osel_d)
        nc.sync.dma_start(ones32[:], ones32_d)
        nc.sync.dma_start(ones1x128[:], ones1x128_d)
        nc.sync.dma_start(cb[:], cb_d)

        # ---- a = W1 @ v_abs  [32, N] ----
        a_sb = cpool.tile([32, N], FP, tag="a_sb")
        for h in range(2):
            pa = ps_small.tile([32, 512], FP, tag="small", name=f"pa{h}")
            nc.tensor.matmul(pa[:], w1t[:], va[:, h * 512:(h + 1) * 512],
                             start=True, stop=True)
            nc.scalar.copy(a_sb[:, h * 512:(h + 1) * 512], pa[:])

        # ---- A4: a replicated on 4 partition blocks [128, N] ----
        A4 = cpool.tile([128, N], FP, tag="A4")
        for r in range(4):
            nc.sync.dma_start(A4[32 * r:32 * r + 32, :], a_sb[:])

        # ---- S[32r+o, t] = a[o, 4t+r] + b1[o];  S2 = S - 2*b1 ----
        S = cpool.tile([128, G], FP, tag="S")
        S2 = cpool.tile([128, G], FP, tag="S2")
        negS2 = cpool.tile([128, G], FP, tag="negS2")
        a_rows = a_sb[:, 0:RPC].rearrange("p (t r) -> p r t", r=4)
        for r in range(4):
            nc.sync.dma_start(S[32 * r:32 * r + 32, :], a_rows[:, r, :])
        nc.vector.tensor_scalar(out=S[:], in0=S[:], scalar1=b1r[:], scalar2=None,
                                op0=ALU.add)
        nc.vector.tensor_scalar(out=S2[:], in0=S[:], scalar1=b1r[:], scalar2=b1r[:],
                                op0=ALU.subtract, op1=ALU.subtract)
        nc.vector.tensor_scalar(out=negS2[:], in0=S2[:], scalar1=-1.0, scalar2=None,
                                op0=ALU.mult)

        if dbg is not None:
            nc.sync.dma_start(dbg["dbg_a"], a_sb[:])
            nc.sync.dma_start(dbg["dbg_S"], S[:])
            nc.sync.dma_start(dbg["dbg_S2"], S2[:])

        # ---- pass A: held tiles hold NEGATED h = min(A4 - S, 0) = -relu(...).
        # Sum h via PE (osel contracts partitions (r,o)->o, PSUM accumulates
        # over groups); sum h^2 via ACT Square accum_out (HW drops the second
        # ALU op of tensor_scalar when accum is enabled, so no DVE accum). ----
        sumsq = cpool.tile([128, G], FP, tag="sumsq")  # sum of h^2
        sh0 = ps_stat.tile([32, 512], FP, tag="sh0", name="sh0")
        sh1 = ps_stat.tile([32, 512], FP, tag="sh1", name="sh1")
        atiles = []
        for t in range(G):
            at = apool.tile([128, N], FP, tag="atile")
            nc.vector.tensor_scalar(out=at[:], in0=A4[:], scalar1=S[:, t:t + 1],
                                    scalar2=0.0, op0=ALU.subtract, op1=ALU.min)
            sq = sqpool.tile([128, N], FP, tag="sq")
            nc.scalar.activation(sq[:], at[:], AF.Square,
                                 accum_out=sumsq[:, t:t + 1])
            nc.tensor.matmul(sh0[:], osel[:], at[:, 0:512],
                             start=(t == 0), stop=(t == G - 1))
            nc.tensor.matmul(sh1[:], osel[:], at[:, 512:N],
                             start=(t == 0), stop=(t == G - 1))
            atiles.append(at)

        # ---- stats: per-(r,o) -> per-channel -> global allreduce ----
        red_sq = cpool.tile([128, 1], FP, tag="red_sq")
        nc.vector.reduce_sum(red_sq[:], sumsq[:], axis=AX.X)
        if dbg is not None:
            nc.sync.dma_start(dbg["dbg_red2"], red_sq[:])
        pst = ps_small.tile([32, 512], FP, tag="small", name="pst")
        nc.tensor.matmul(pst[:, 0:1], osel[:], red_sq[:], start=True, stop=True)
        shsum = cpool.tile([32, 2], FP, tag="shsum")
        nc.vector.reduce_sum(shsum[:, 0:1], sh0[:], axis=AX.X)
        nc.vector.reduce_sum(shsum[:, 1:2], sh1[:], axis=AX.X)
        st_loc = cpool.tile([32, 2], FP, tag="st_loc")
        nc.vector.tensor_add(out=st_loc[:, 0:1], in0=shsum[:, 0:1],
                             in1=shsum[:, 1:2])
        nc.scalar.copy(st_loc[:, 1:2], pst[:, 0:1])
        cc_in = dpool.tile([32, 2], FP, tag="cc_in")
        cc_out = dpool.tile([32, 2], FP, tag="cc_out")
        nc.sync.dma_start(cc_in[:], st_loc[:])
        nc.gpsimd.collective_compute(
            "AllReduce", ALU.add, replica_groups=[list(range(cc_cores))],
            ins=[cc_in.opt()], outs=[cc_out.opt()],
        )
        st_glob = cpool.tile([32, 2], FP, tag="st_glob")
        nc.sync.dma_start(st_glob[:], cc_out[:])

        # ---- BN math on [32,1] ----
        inv_n2 = 1.0 / float(N * N)
        mu = cpool.tile([32, 1], FP, tag="mu")
        nc.scalar.mul(mu[:], st_glob[:, 0:1], -inv_n2)  # sumh held -h
        ex2 = cpool.tile([32, 1], FP, tag="ex2")
        nc.scalar.mul(ex2[:], st_glob[:, 1:2], inv_n2)
        mu2 = cpool.tile([32, 1], FP, tag="mu2")
        nc.scalar.square(mu2[:], mu[:])
        varp = cpool.tile([32, 1], FP, tag="varp")
        nc.vector.tensor_sub(out=varp[:], in0=ex2[:], in1=mu2[:])
        nc.vector.tensor_scalar(out=varp[:], in0=varp[:], scalar1=EPS,
                                scalar2=None, op0=ALU.add)
        rcp = cpool.tile([32, 1], FP, tag="rcp")
        nc.vector.reciprocal(rcp[:], varp[:])
        rs = cpool.tile([32, 1], FP, tag="rs")
        nc.scalar.sqrt(rs[:], rcp[:])
        wt = cpool.tile([32, 1], FP, tag="wt")
        nc.vector.tensor_mul(out=wt[:], in0=w2g[:], in1=rs[:])
        wmu = cpool.tile([32, 1], FP, tag="wmu")
        nc.vector.tensor_mul(out=wmu[:], in0=wt[:], in1=mu[:])
        pc = ps_small.tile([32, 512], FP, tag="small", name="pc")
        nc.tensor.matmul(pc[0:1, 0:1], wmu[:], ones32[:], start=True, stop=True)
        c11 = cpool.tile([1, 1], FP, tag="c11")
        nc.scalar.activation(c11[:], pc[0:1, 0:1], AF.Identity, bias=cb[:], scale=-1.0)
        pbc = ps_small.tile([128, 512], FP, tag="small", name="pbc")
        nc.tensor.matmul(pbc[:, 0:1], ones1x128[:], c11[:], start=True, stop=True)
        C128 = cpool.tile([128, 1], FP, tag="C128")
        nc.scalar.copy(C128[:], pbc[:, 0:1])

        if dbg is not None:
            nc.sync.dma_start(dbg["dbg_st"], st_glob[:])
            nc.sync.dma_start(dbg["dbg_wt"], wt[:])
            nc.sync.dma_start(dbg["dbg_C"], C128[:])

        # ---- Lfat: cols 124+r hold wt at partitions 32r..32r+31.  The lhsT
        # for group t is the 128-col window starting at col 124-4t, which
        # places wt[o] at (32r+o, out-row 4t+r); every matmul writes the full
        # [128, 512] PSUM tile (zeros off-group), accumulating across t. ----
        Lfat = cpool.tile([128, 252], FP, tag="Lfat")
        nc.vector.memset(Lfat[:], 0.0)
        for r in range(4):
            nc.sync.dma_start(Lfat[32 * r:32 * r + 32, 124 + r:125 + r], wt[:])

        if dbg is not None:
            nc.sync.dma_start(dbg["dbg_L"], Lfat[:])
            nc.sync.dma_start(dbg["dbg_at0"], atiles[0][:])

        # ---- pass B: matmuls over held (negated) D1 tiles + fresh D2 tiles ----
        pD = [ps_main.tile([128, 512], FP, tag=f"pD{i}", name=f"pD{i}")
              for i in range(4)]
        for t in range(G):
            lhs = Lfat[:, 124 - 4 * t:252 - 4 * t]
            st, sp = (t == 0), (t == G - 1)
            nc.tensor.matmul(pD[0][:], lhs, atiles[t][:, 0:512],
                             start=st, stop=sp)
            nc.tensor.matmul(pD[1][:], lhs, atiles[t][:, 512:N],
                             start=st, stop=sp)
            d2t = d2pool.tile([128, N], FP, tag="d2t")
            if t % 8 == 7:
                nc.scalar.activation(d2t[:], A4[:], AF.Relu,
                                     bias=negS2[:, t:t + 1], scale=1.0)
            else:
                nc.vector.tensor_scalar(out=d2t[:], in0=A4[:],
                                        scalar1=S2[:, t:t + 1], scalar2=0.0,
                                        op0=ALU.subtract, op1=ALU.max)
            if dbg is not None and t == 0:
                nc.sync.dma_start(dbg["dbg_d2t0"], d2t[:])
            nc.tensor.matmul(pD[2][:], lhs, d2t[:, 0:512],
                             start=st, stop=sp)
            nc.tensor.matmul(pD[3][:], lhs, d2t[:, 512:N],
                             start=st, stop=sp)

        # ---- exp + symmetrize (0.5 folded into C via ln(0.5)) ----
        d1 = opool.tile([128, N], FP, tag="d1")
        d2 = opool.tile([128, N], FP, tag="d2")
        dist = opool.tile([128, N], FP, tag="dist")
        # D1 psums hold -(d_pre - C): exp(scale=-1) undoes the negation
        nc.scalar.activation(d1[:, 0:512], pD[0][:], AF.Exp, bias=C128[:], scale=-1.0)
        nc.scalar.activation(d1[:, 512:N], pD[1][:], AF.Exp, bias=C128[:], scale=-1.0)
        nc.scalar.activation(d2[:, 0:512], pD[2][:], AF.Exp, bias=C128[:], scale=1.0)
        nc.scalar.activation(d2[:, 512:N], pD[3][:], AF.Exp, bias=C128[:], scale=1.0)
        if dbg is not None:
            nc.sync.dma_start(dbg["dbg_d1"], d1[:])
            nc.sync.dma_start(dbg["dbg_d2"], d2[:])
        nc.vector.tensor_add(out=dist[:], in0=d1[:], in1=d2[:])
        nc.sync.dma_start(dist_d, dist[:])


def _host_inputs(v_abs, W1, b1, gamma, beta, W2, b2):
    va16 = np.ascontiguousarray(v_abs.reshape(16, N).astype(np.float32))
    w1t = np.ascontiguousarray(W1.T.astype(np.float32))
    b1rep = np.tile(b1.astype(np.float32), 4)[:, None]
    w2g = (W2[0] * gamma).astype(np.float32)[:, None]
    osel = np.zeros((128, 32), np.float32)
    for r in range(4):
        osel[32 * r + np.arange(32), np.arange(32)] = 1.0
    ones32 = np.ones((32, 1), np.float32)
    ones1x128 = np.ones((1, 128), np.float32)
    cb = np.array([[float((W2[0] * beta).sum() + b2[0] + np.log(0.5))]], np.float32)
    common = dict(w1t=w1t, b1rep=b1rep, w2g=w2g, osel=osel, ones32=ones32,
                  ones1x128=ones1x128, cb=cb)
    in_maps = []
    for c in range(NCORES):
        m = dict(common)
        m["va"] = np.ascontiguousarray(np.roll(va16, -RPC * c, axis=1))
        in_maps.append(m)
    return in_maps


def run_device(v_abs, W1, b1, gamma, beta, W2, b2, trace=False):
    """Returns (dist_full [N,N], BassKernelResults)."""
    if "nc" not in _CACHE:
        _CACHE["nc"] = _build()
    nc = _CACHE["nc"]
    in_maps = _host_inputs(v_abs, W1, b1, gamma, beta, W2, b2)
    res = run_bass_kernel_spmd(nc, in_maps, core_ids=list(range(NCORES)),
                               trace=trace)
    dist = np.empty((N, N), np.float32)
    for c in range(NCORES):
        dist[RPC * c:RPC * (c + 1), :] = np.roll(res.results[c]["dist"], RPC * c,
                                                 axis=1)
    return dist, res


def _find_group_indices(dist):
    E = dist <= TH
    E[np.triu_indices(N)] = False
    labels = np.arange(N)
    for r in range(N):
        row = E[r]
        idx = np.nonzero(row)[0]
        if idx.size == 0:
            continue
        cmax = idx[-1]
        mask = row.copy()
        mask[cmax] = False
        mask[r] = True
        labels = np.where(mask[labels], cmax, labels)
    present = np.zeros(N, bool)
    present[labels] = True
    rank = np.cumsum(present) - 1
    return rank[labels].astype(np.int32)


def kernel(v, v_abs, W1, b1, gamma, beta, W2, b2):
    dist, _ = run_device(v_abs, W1, b1, gamma, beta, W2, b2, trace=False)
    indices = _find_group_indices(dist)
    # hard=True: forward value of stop_gradient(v - v_soft) + v_soft is v
    v_out = np.asarray(v, dtype=np.float32).copy()
    return (v_out, indices)
